# revision 68
# baseline (speedup 1.0000x reference)
"""Trainium2 Bass kernel for nn_PlaneRefineBatch (stereo undistort + DLT
triangulation + reprojection error).

Contract: kernel(**inputs) takes the FULL inputs (camera_points [16,131072,2],
projector_points [16,131072,2], R [3,3], T [3], camera_mtx, camera_dist,
projector_mtx, projector_dist) and returns (pts [16,131072,3] f32, error f32
scalar) exactly like the reference.

Strategy: pure data-parallel over the 16-batch axis: 2 batches per core on 8
cores (SPMD, one Bass program). Per core 262144 points = [128 partitions x
2048 free] fp32 planes, whole-plane instructions. Calibration scalars are
baked into the program as immediates (program cached per calibration). Heavy
math on the Vector engine (fused scalar_tensor_tensor / ln_bwd_dx ops,
approx reciprocals); the error path's squares/sqrt+row-sum on the Scalar
engine.

The pipeline is emitted by `_emit_pipeline` against an abstract backend so
the exact same op sequence runs as Bass IR (class _BassB) or as a bit-
faithful numpy fp32 emulation (class _NpB in emulate.py) for fast numerical
iteration against the jax reference.

Math notes:
 - undistort: 2 fixed-point iterations match the reference's 5 within fp32
   noise (contraction ~0.03/iter); 1/radial = 1-g+g^2 exactly to O(g^3)~1e-11.
 - triangulation: T_MODE="exact" replicates the reference's einsum rounding
   (m3/m4/b rows, then ordered sums of products); "feat" uses cheaper linear
   combos of (s, x2, y2, x1, y1, sc).
 - solve: LDL (AtA+eps*I is SPD); reciprocal flavor per RECIP_MODE.
 - reprojection feeds only the scalar error (a mean, which overflows to +inf
   on degenerate points exactly like the reference does).
"""
import numpy as np
from contextlib import ExitStack

P = 128           # SBUF partitions
F = 2048          # free-dim elements per partition per plane
NCORES = 8
PER_CORE = P * F  # 262144 points per core
NREG = 17         # register planes of [P, F] fp32

UNDIST_ITERS = 2
T_MODE = "exact"       # "exact" (replicate einsum rounding) | "feat"
RECIP_MODE = "exact"   # LDL pivots: "exact" | "acc" (2 ULP) | "fast" (51 ULP)
# P_SLIM drops the reprojection's tangential terms + uses the fast 1/z.
# For the canonical setup_inputs() the error is +inf either way, but keep
# the faithful path so the error is also right for any finite-error inputs.
P_SLIM = False
# Reuse iteration-1's 1/radial in iteration 2 (it moves by ~1 ulp of x
# between iterations; verified against the reference in the emulator).
REUSE_G = False

_CACHE = {}
_CUSTOM_OPS = None


def _register_custom_ops():
    """Register fused DVE ops with the custom-DVE registry (HW-verified
    bitwise against the discrete chains they replace, incl. inf overflow):
      HORNER3: ((in0*s0 + s1)*in0 + imm2)*in0        (radial cubic)
      SQSQ:    (in0*in0 + in1*in1) + s0              (r2 / squared distance)
      UX2:     (in0 + in1*in1*s0)*s1                 ((r2 + 2x^2)*p)
    """
    global _CUSTOM_OPS
    if _CUSTOM_OPS is not None:
        return _CUSTOM_OPS
    from concourse import dve_ops
    from concourse.dve_spec import (Spec, Src0, Src1, C0, C1, C2, lower,
                                    _has_src1)
    from concourse.dve_uop import DveOpSpec
    f32 = np.float32

    def _ref_h3(in0, in1, s0, s1, imm2):
        t = (in0.astype(f32) * f32(s0)).astype(f32)
        t = (t + f32(s1)).astype(f32)
        t = (t * in0).astype(f32)
        t = (t + f32(imm2)).astype(f32)
        return (t * in0).astype(f32)

    def _ref_sqsq(in0, in1, s0, s1, imm2):
        t = (in0.astype(f32) * in0.astype(f32)).astype(f32)
        u = (in1.astype(f32) * in1.astype(f32)).astype(f32)
        return ((t + u).astype(f32) + f32(s0)).astype(f32)

    def _ref_ux2(in0, in1, s0, s1, imm2):
        t = (in1.astype(f32) * in1.astype(f32)).astype(f32)
        u = (t * f32(s0)).astype(f32)
        v = (in0.astype(f32) + u).astype(f32)
        return (v * f32(s1)).astype(f32)

    def _ref_tsts(in0, in1, s0, s1, imm2):
        t = (in0.astype(f32) * in1.astype(f32)).astype(f32)
        u = (t * f32(s0)).astype(f32)
        return (u + f32(s1)).astype(f32)

    defs = [
        ("HORNER3_ANT", Spec(body=((Src0 * C0 + C1) * Src0 + C2) * Src0,
                             reference=_ref_h3)),
        ("SQSQ_ANT", Spec(body=(Src0 * Src0 + Src1 * Src1) + C0,
                          reference=_ref_sqsq)),
        ("UX2_ANT", Spec(body=(Src0 + Src1 * Src1 * C0) * C1,
                         reference=_ref_ux2)),
        ("TSTS_ANT", Spec(body=Src0 * Src1 * C0 + C1,
                          reference=_ref_tsts)),
    ]
    ops = {}
    for name, spec in defs:
        if name in dve_ops._SUB_OPCODE_FOR_NAME:
            ops[name] = next(o for o in dve_ops.OPS if o.name == name)
            continue
        op = dve_ops.DveOp(name, spec, subdim=False, uops_sha={})
        row = max(dve_ops._SUB_OPCODE_FOR_NAME.values()) + 1
        assert row < 0x20
        dve_ops.OPS.append(op)
        dve_ops.CUSTOM_DVE_SPECS[name] = spec
        dve_ops._SUB_OPCODE_FOR_NAME[name] = row
        for ver in ("v3", "v4"):
            r = DveOpSpec(name=name, opcode=row, uops=lower(spec, ver=ver),
                          rd1_en=_has_src1(spec))
            op.uops_sha[ver] = r.sha(ver)
        ops[name] = op
    _CUSTOM_OPS = ops
    return ops


def _params_key(*arrs):
    return (UNDIST_ITERS, T_MODE, RECIP_MODE, P_SLIM) + tuple(a.tobytes() for a in arrs)


# --------------------------------------------------------------------------
# Shared pipeline emission (backend-agnostic)
# --------------------------------------------------------------------------

def _emit_undistort(B, u, v, K, dist, iters, split_head=False):
    fx, fy, cx, cy = (float(K[0, 0]), float(K[1, 1]),
                      float(K[0, 2]), float(K[1, 2]))
    k1, k2, p1, p2, k3 = [float(x) for x in dist]
    if split_head:
        # column-half normalize so the first half runs while the second
        # half of the input DMA is still streaming (value-identical)
        x0 = B.ts_halves(u, cx, 1.0 / fx, "subtract", "mult")
        y0 = B.ts_halves(v, cy, 1.0 / fy, "subtract", "mult")
    else:
        x0 = B.ts(u, cx, 1.0 / fx, "subtract", "mult")
        y0 = B.ts(v, cy, 1.0 / fy, "subtract", "mult")
    x, y = x0, y0
    ir_saved = None
    for it in range(iters):
        last = it == iters - 1
        r2 = B.sqsq(x, y, 0.0)              # x^2+y^2, fused
        xy = B.tt(x, y, "mult")
        ux = B.ux2(r2, x, 2.0, p2)          # p2*(r2+2x^2), fused
        uy = B.ux2(r2, y, 2.0, p1)          # p1*(r2+2y^2), fused
        if it > 0:
            B.rel(x, y)
        if REUSE_G and ir_saved is not None:
            ir = ir_saved
            B.rel(r2)
            wx = B.stt(xy, 2.0 * p1, ux, "mult", "add")
            wy = B.stt(xy, 2.0 * p2, uy, "mult", "add")
        else:
            g = B.horner3(r2, k3, k2, k1)  # ((k3*r2+k2)*r2+k1)*r2
            B.rel(r2)
            gsq = B.sq(g)   # ACT; covered by the DVE w-ops below
            wx = B.stt(xy, 2.0 * p1, ux, "mult", "add")
            wy = B.stt(xy, 2.0 * p2, uy, "mult", "add")
            ir = B.stt(gsq, 1.0, g, "add", "subtract")  # 1+g^2-g
            B.rel(g, gsq)
            if REUSE_G and not last:
                ir_saved = ir
        B.rel(ux, uy, xy)
        tx = B.tt(x0, wx, "subtract")
        ty = B.tt(y0, wy, "subtract")
        B.rel(wx, wy)
        x = B.tt(tx, ir, "mult")
        y = B.tt(ty, ir, "mult")
        B.rel(tx, ty)
        if not (REUSE_G and not last):
            B.rel(ir)
    B.rel(x0, y0)
    return x, y


def _emit_lin(B, terms, const, dst=None):
    """sum(c_i * t_i) + const via ln_bwd chains; drops zero coefficients."""
    terms = [(float(c), t) for c, t in terms if float(c) != 0.0]
    const = float(const)
    assert terms
    if len(terms) == 1:
        c0, t0 = terms[0]
        return B.ts(t0, c0, const, "mult", "add", dst=dst)
    terms.sort(key=lambda ct: -abs(ct[0]))
    (c0, t0), (c1, t1) = terms[0], terms[1]
    if len(terms) == 2:
        return B.lnb(t0, t1, -c1 / c0, -const / c0, c0, dst=dst)
    acc = B.lnb(t0, t1, -c1 / c0, 0.0, c0)
    for i, (ci, ti) in enumerate(terms[2:]):
        last = i == len(terms) - 3
        nxt = B.lnb(acc, ti, -ci, -const if last else 0.0, 1.0,
                    dst=(dst if last else None))
        B.rel(acc)
        acc = nxt
    return acc


def _emit_normal_eqs_feat(B, x1, y1, x2, y2, Rd, Td):
    """AtA/-Atb as linear combos of (s, x2, y2, x1, y1, sc)."""
    a_r, b_r, c_r = Rd[0], Rd[1], Rd[2]
    C2m = np.outer(c_r, c_r)
    CA = np.outer(a_r, c_r) + np.outer(c_r, a_r)
    CB = np.outer(b_r, c_r) + np.outer(c_r, b_r)
    Dm = np.outer(a_r, a_r) + np.outer(b_r, b_r)
    EPS = 1e-9
    sx2 = B.sq(x2); sy2 = B.sq(y2)
    s = B.tt(sx2, sy2, "add"); B.rel(sx2, sy2)
    sx1 = B.sq(x1); sy1 = B.sq(y1)
    sc = B.tt(sx1, sy1, "add"); B.rel(sx1, sy1)
    a00 = _emit_lin(B, [(C2m[0, 0], s), (-CA[0, 0], x2), (-CB[0, 0], y2)],
                    Dm[0, 0] + 1.0)
    a01 = _emit_lin(B, [(C2m[0, 1], s), (-CA[0, 1], x2), (-CB[0, 1], y2)],
                    Dm[0, 1])
    t11 = [(C2m[1, 1], s), (-CA[1, 1], x2), (-CB[1, 1], y2)]
    a11c = Dm[1, 1] + 1.0
    a11 = None if all(float(c) == 0.0 for c, _ in t11) else \
        _emit_lin(B, t11, a11c)
    a02 = _emit_lin(B, [(C2m[0, 2], s), (-CA[0, 2], x2), (-CB[0, 2], y2),
                        (-1.0, x1)], Dm[0, 2])
    a12 = _emit_lin(B, [(C2m[1, 2], s), (-CA[1, 2], x2), (-CB[1, 2], y2),
                        (-1.0, y1)], Dm[1, 2])
    a22 = _emit_lin(B, [(C2m[2, 2], s), (-CA[2, 2], x2), (-CB[2, 2], y2),
                        (1.0, sc)], Dm[2, 2] + EPS)
    B.rel(sc, x1, y1)
    q0 = _emit_lin(B, [(-Td[2] * c_r[0], s),
                       (c_r[0] * Td[0] + a_r[0] * Td[2], x2),
                       (c_r[0] * Td[1] + b_r[0] * Td[2], y2)],
                   -(a_r[0] * Td[0] + b_r[0] * Td[1]))
    q1 = _emit_lin(B, [(-Td[2] * c_r[1], s),
                       (c_r[1] * Td[0] + a_r[1] * Td[2], x2),
                       (c_r[1] * Td[1] + b_r[1] * Td[2], y2)],
                   -(a_r[1] * Td[0] + b_r[1] * Td[1]))
    q2 = _emit_lin(B, [(-Td[2] * c_r[2], s),
                       (c_r[2] * Td[0] + a_r[2] * Td[2], x2),
                       (c_r[2] * Td[1] + b_r[2] * Td[2], y2)],
                   -(a_r[2] * Td[0] + b_r[2] * Td[1]))
    B.rel(s, x2, y2)
    return a00, a01, a11, a11c, a02, a12, a22, q0, q1, q2


def _emit_normal_eqs_exact(B, x1, y1, x2, y2, Rd, Td):
    """Replicate the reference's einsum rounding: m3 = x2*R2 - R0,
    m4 = y2*R2 - R1, nb3 = -(x2*T2 - T0), nb4 = -(y2*T2 - T1), then AtA
    entries as ((m1.m1 + m2.m2) + m3.m3) + m4.m4 in that order, and
    q = -Atb (negation exact). Values are (tensor | ('c', float)) with
    constant folding, so zero rows of R/T cost nothing."""
    f32 = np.float32

    def is_c(v):
        return isinstance(v, tuple) and v[0] == 'c'

    def row(t, c, a):
        # t*c - a; c == 0 -> elementwise (+-0 - a) = -a exactly
        if float(c) == 0.0:
            return ('c', float(f32(-f32(a))))
        return B.ts(t, float(c), float(a), "mult", "subtract")

    def nrow(t, c, a):
        # -(t*c - a) = t*(-c) + a
        if float(c) == 0.0:
            return ('c', float(f32(a)))
        return B.ts(t, float(-c), float(a), "mult", "add")

    def mul(x, y):
        if is_c(x) and is_c(y):
            return ('c', float(f32(x[1]) * f32(y[1])))
        if is_c(y):
            x, y = y, x
        if is_c(x):
            if x[1] == 0.0:
                return ('c', 0.0)
            return B.ts(y, float(x[1]), 1.0, "mult", "mult")
        return B.tt(x, y, "mult")

    def sqv(x):
        if is_c(x):
            return ('c', float(f32(x[1]) * f32(x[1])))
        return B.sq(x)

    def add(x, y):
        if is_c(x) and is_c(y):
            return ('c', float(f32(x[1]) + f32(y[1])))
        if is_c(y):
            x, y = y, x
        if is_c(x):
            if x[1] == 0.0:
                return y
            return B.ts(y, float(x[1]), 1.0, "add", "mult")
        return B.tt(x, y, "add")

    def neg(x):
        if is_c(x):
            return ('c', -x[1])
        return B.ts(x, -1.0, 1.0, "mult", "mult")

    def relv(*vs):
        B.rel(*[v for v in vs if not is_c(v)])

    a_r, b_r, c_r = Rd[0], Rd[1], Rd[2]
    m3 = [row(x2, c_r[j], a_r[j]) for j in range(3)]
    m4 = [row(y2, c_r[j], b_r[j]) for j in range(3)]
    nb3 = nrow(x2, Td[2], Td[0])
    nb4 = nrow(y2, Td[2], Td[1])
    relv(x2, y2)  # fully consumed by the m/nb rows

    def fin(res, temps):
        seen = {id(res)}
        for v in temps:
            if v is None or is_c(v) or id(v) in seen:
                continue
            seen.add(id(v))
            B.rel(v)
        return res

    def entry(base_c, p, q):
        # ((base_c + p) + q) with reference rounding order
        t = add(('c', base_c), p)
        return fin(add(t, q), [p, q, t])

    def entry_neg(xb, p, q):
        # ((-xb + p) + q); xb is a live tensor (not freed here)
        if is_c(p):
            assert not is_c(q)
            if p[1] == 0.0:
                return B.tt(q, xb, "subtract")  # (-xb + 0) + q == q - xb
            t = B.ts(xb, -1.0, float(p[1]), "mult", "add")  # -xb + p
            return fin(add(t, q), [q, t])
        t = B.tt(p, xb, "subtract")         # == -xb + p
        return fin(add(t, q), [p, q, t])

    # hoist ACT squares just enough to overlap the DVE products below
    # without blowing the register budget
    s3x, s4x = sqv(m3[0]), sqv(m4[0])
    s3y, s4y = sqv(m3[1]), sqv(m4[1])
    a01 = entry(0.0, mul(m3[0], m3[1]), mul(m4[0], m4[1]))
    a02 = entry_neg(x1, mul(m3[0], m3[2]), mul(m4[0], m4[2]))
    a12 = entry_neg(y1, mul(m3[1], m3[2]), mul(m4[1], m4[2]))
    sx1 = sqv(x1); sy1 = sqv(y1)
    relv(x1, y1)  # dead once their squares are issued
    a00 = entry(1.0, s3x, s4x)
    a11 = entry(1.0, s3y, s4y)
    s3z, s4z = sqv(m3[2]), sqv(m4[2])
    sc = add(sx1, sy1); relv(sx1, sy1)
    t = add(sc, s3z); relv(sc, s3z)
    u = add(t, s4z); relv(s4z, t)
    a22 = B.ts(u, 1e-9, 1.0, "add", "mult")
    relv(u)
    qs = []
    for j in range(3):
        pj = mul(m3[j], nb3)
        qj = mul(m4[j], nb4)
        qs.append(fin(add(pj, qj), [pj, qj]))
    relv(*m3, *m4, nb3, nb4)
    a11c = None
    if is_c(a11):
        a11c = a11[1]
        a11 = None
    for v in (a00, a01, a02, a12, a22, qs[0], qs[1], qs[2]):
        assert not is_c(v), "unexpected constant normal-equation entry"
    return a00, a01, a11, a11c, a02, a12, a22, qs[0], qs[1], qs[2]


def _emit_solve_ldl(B, a00, a01, a11, a11c, a02, a12, a22, q0, q1, q2):
    """LDL solve of (AtA)x = q, writing x into B.outX/outY/outZ."""
    i0 = B.recip(a00, which=0)
    B.rel(a00)
    l10 = B.tt(a01, i0, "mult")
    l20 = B.tt(a02, i0, "mult")
    t3 = B.tt(l20, a02, "mult")
    B.rel(a02)
    # NOTE: fusing this into a custom TSTS op (a*b*c0+c1) coincided with an
    # NRT_EXEC_UNIT_UNRECOVERABLE device crash -- left unfused.
    t = B.tt(l10, a01, "mult")
    if a11 is None:
        d1v = B.ts(t, -1.0, float(a11c), "mult", "add")
    else:
        d1v = B.tt(a11, t, "subtract")
        B.rel(a11)
    B.rel(t)
    t2 = B.tt(l20, a01, "mult")
    B.rel(a01)
    a12p = B.tt(a12, t2, "subtract")
    B.rel(a12, t2)
    i1 = B.recip(d1v, which=1)
    B.rel(d1v)
    l21 = B.tt(a12p, i1, "mult")
    t4 = B.tt(l21, a12p, "mult")
    B.rel(a12p)
    t5 = B.tt(t3, t4, "add")
    B.rel(t3, t4)
    d2v = B.tt(a22, t5, "subtract")
    B.rel(a22, t5)
    i2 = B.recip(d2v, which=2)
    B.rel(d2v)
    t6 = B.tt(l10, q0, "mult")
    c1 = B.tt(q1, t6, "subtract")
    B.rel(q1, t6)
    t7 = B.tt(l20, q0, "mult")
    t8 = B.tt(q2, t7, "subtract")
    B.rel(q2, t7)
    t9 = B.tt(l21, c1, "mult")
    c2 = B.tt(t8, t9, "subtract")
    B.rel(t8, t9)
    e0 = B.tt(q0, i0, "mult")
    B.rel(q0, i0)
    e1 = B.tt(c1, i1, "mult")
    B.rel(c1, i1)
    B.tt(c2, i2, "mult", dst=B.outZ)
    B.rel(c2, i2)
    t10 = B.tt(l21, B.outZ, "mult")
    B.rel(l21)
    B.tt(e1, t10, "subtract", dst=B.outY)
    B.rel(e1, t10)
    t11 = B.tt(l10, B.outY, "mult")
    B.rel(l10)
    t12 = B.tt(e0, t11, "subtract")
    B.rel(e0, t11)
    t13 = B.tt(l20, B.outZ, "mult")
    B.rel(l20)
    B.tt(t12, t13, "subtract", dst=B.outX)
    B.rel(t12, t13)


def _emit_reproject_view(B, X, Y, Z, uo, vo, K, dist, slot, filler=None):
    fx, fy, cx, cy = (float(K[0, 0]), float(K[1, 1]),
                      float(K[0, 2]), float(K[1, 2]))
    k1, k2, p1, p2, k3 = [float(x) for x in dist]
    iz = B.recip_err(Z)
    x = B.tt(X, iz, "mult")
    y = B.tt(Y, iz, "mult")
    B.rel(iz)
    r2 = B.sqsq(x, y, 0.0)
    if not P_SLIM:
        xy = B.tt(x, y, "mult")
        ux = B.ux2(r2, x, 2.0, p2)
        uy = B.ux2(r2, y, 2.0, p1)
    g = B.horner3(r2, k3, k2, k1)
    B.rel(r2)
    sx = B.stt(g, 1.0, x, "add", "mult")   # radial*x
    sy = B.stt(g, 1.0, y, "add", "mult")
    B.rel(g, x, y)
    if P_SLIM:
        xd, yd = sx, sy
    else:
        t1 = B.stt(xy, 2.0 * p1, ux, "mult", "add")
        t2 = B.stt(xy, 2.0 * p2, uy, "mult", "add")
        B.rel(ux, uy, xy)
        xd = B.tt(sx, t1, "add")
        yd = B.tt(sy, t2, "add")
        B.rel(sx, sy, t1, t2)
    du = B.lnb(xd, uo, 1.0 / fx, -cx / fx, fx)  # fx*xd+cx-uo
    dv = B.lnb(yd, vo, 1.0 / fy, -cy / fy, fy)
    B.rel(xd, yd)
    out = filler() if filler is not None else None
    # (du^2 + dv^2) + eps fused -- matches the reference's sum-then-eps order
    d2s = B.sqsq(du, dv, 1e-9)
    B.rel(du, dv)
    B.sqrt_accum(d2s, slot)
    B.rel(d2s)
    return out


def _emit_pipeline(B, R, T, K1, d1, K2, d2):
    Rd = np.asarray(R, np.float64)
    Td = np.asarray(T, np.float64)
    # NOTE: column-splitting the input DMA + first normalize (to shave the
    # ~3 us DMA head) modeled fine but crashed real HW with
    # NRT_EXEC_UNIT_UNRECOVERABLE -- do not re-enable split_head without
    # hardware validation.
    x1, y1 = _emit_undistort(B, B.u1, B.v1, K1, d1, UNDIST_ITERS)
    B.prefetch_prj()   # projector input DMA off the camera DMA's head
    x2, y2 = _emit_undistort(B, B.u2, B.v2, K2, d2, UNDIST_ITERS)
    if T_MODE == "exact":
        eqs = _emit_normal_eqs_exact(B, x1, y1, x2, y2, Rd, Td)
    else:
        eqs = _emit_normal_eqs_feat(B, x1, y1, x2, y2, Rd, Td)
    _emit_solve_ldl(B, *eqs)
    B.pts_ready()

    trivial_py = (Rd[1, 0] == 0.0 and Rd[1, 2] == 0.0
                  and Rd[1, 1] == 1.0 and Td[1] == 0.0)

    def transform():
        # projector-frame transform; emitted as DVE filler inside the
        # camera view's ACT-square window
        px = _emit_lin(B, [(Rd[0, 0], B.outX), (Rd[0, 1], B.outY),
                           (Rd[0, 2], B.outZ)], Td[0])
        py = B.outY if trivial_py else _emit_lin(
            B, [(Rd[1, 0], B.outX), (Rd[1, 1], B.outY), (Rd[1, 2], B.outZ)],
            Td[1])
        pz = _emit_lin(B, [(Rd[2, 0], B.outX), (Rd[2, 1], B.outY),
                           (Rd[2, 2], B.outZ)], Td[2])
        return px, py, pz

    px, py, pz = _emit_reproject_view(B, B.outX, B.outY, B.outZ,
                                      B.u1, B.v1, K1, d1, 0,
                                      filler=transform)
    _emit_reproject_view(B, px, py, pz, B.u2, B.v2, K2, d2, 1)
    B.rel(px, pz)
    if not trivial_py:
        B.rel(py)


# --------------------------------------------------------------------------
# Bass backend
# --------------------------------------------------------------------------

class _BassB:
    def __init__(self):
        from concourse import mybir, tile
        from concourse.bacc import Bacc
        self.mybir = mybir
        self.f32 = mybir.dt.float32
        self.Alu = mybir.AluOpType
        self.Act = mybir.ActivationFunctionType
        nc = Bacc("TRN2", target_bir_lowering=False, debug=False,
                  num_devices=NCORES)
        self.nc = nc
        self.cam_d = nc.dram_tensor("cam", [PER_CORE, 2], self.f32,
                                    kind="ExternalInput").ap()
        self.prj_d = nc.dram_tensor("prj", [PER_CORE, 2], self.f32,
                                    kind="ExternalInput").ap()
        self.pts_d = nc.dram_tensor("pts", [PER_CORE, 3], self.f32,
                                    kind="ExternalOutput").ap()
        self.esum_d = nc.dram_tensor("esum", [P, 2], self.f32,
                                     kind="ExternalOutput").ap()
        self._tile = tile
        self._free = []
        self._nreg = 0
        self._pinned = set()

    def alloc(self):
        if self._free:
            return self._free.pop()
        t = self.pool.tile([P, F], self.f32, tag=f"r{self._nreg}")
        self._nreg += 1
        assert self._nreg <= NREG, "register budget exceeded"
        return t[:]

    def rel(self, *aps):
        for ap in aps:
            if ap is not None and id(ap) not in self._pinned:
                self._free.append(ap)

    def _d(self, dst):
        return dst if dst is not None else self.alloc()

    def tt(self, a, b, op, dst=None):
        d = self._d(dst)
        self.nc.vector.tensor_tensor(d, a, b, getattr(self.Alu, op))
        return d

    def ts(self, a, s1, s2, op0, op1, dst=None):
        d = self._d(dst)
        self.nc.vector.tensor_scalar(d, a, float(s1), float(s2),
                                     getattr(self.Alu, op0),
                                     getattr(self.Alu, op1))
        return d

    def ts_halves(self, a, s1, s2, op0, op1, dst=None):
        # same values as ts(), emitted as two column-half ops so the first
        # half can start as soon as the first input-DMA chunk lands
        d = self._d(dst)
        h = F // 2
        for lo, hi in ((0, h), (h, F)):
            self.nc.vector.tensor_scalar(d[:, lo:hi], a[:, lo:hi],
                                         float(s1), float(s2),
                                         getattr(self.Alu, op0),
                                         getattr(self.Alu, op1))
        return d

    def stt(self, a, s, b, op0, op1, dst=None):
        d = self._d(dst)
        self.nc.vector.scalar_tensor_tensor(d, a, float(s), b,
                                            getattr(self.Alu, op0),
                                            getattr(self.Alu, op1))
        return d

    def lnb(self, a, b, s0, s1, imm2, dst=None):
        d = self._d(dst)
        self.nc.vector.ln_bwd_dx(d, a, b, float(s0), float(s1), float(imm2))
        return d

    def horner3(self, a, c0, c1, c2, dst=None):
        # ((a*c0 + c1)*a + c2)*a in one fused DVE op
        d = self._d(dst)
        self.nc.vector._custom_dve(_register_custom_ops()["HORNER3_ANT"],
                                   out=d, in0=a,
                                   s0=float(c0), s1=float(c1), imm2=float(c2))
        return d

    def tsts(self, a, b, c0, c1, dst=None):
        # a*b*c0 + c1 in one fused DVE op
        d = self._d(dst)
        self.nc.vector._custom_dve(_register_custom_ops()["TSTS_ANT"],
                                   out=d, in0=a, in1=b,
                                   s0=float(c0), s1=float(c1))
        return d

    def sqsq(self, a, b, c0, dst=None):
        # (a*a + b*b) + c0 in one fused DVE op
        d = self._d(dst)
        self.nc.vector._custom_dve(_register_custom_ops()["SQSQ_ANT"],
                                   out=d, in0=a, in1=b, s0=float(c0))
        return d

    def ux2(self, r2, x, c0, c1, dst=None):
        # (r2 + x*x*c0)*c1 in one fused DVE op
        d = self._d(dst)
        self.nc.vector._custom_dve(_register_custom_ops()["UX2_ANT"],
                                   out=d, in0=r2, in1=x,
                                   s0=float(c0), s1=float(c1))
        return d

    def sq(self, a, dst=None):
        # ACT Square is HW-verified bit-exact vs IEEE x*x -> offload the
        # (otherwise DVE-bound) squares to the idle Scalar engine
        return self.sq_act(a, dst=dst)

    def sq_act(self, a, dst=None):
        d = self._d(dst)
        self.nc.scalar.activation(d, a, self.Act.Square)
        return d

    def affine_act(self, a, scale, bias, dst=None):
        d = self._d(dst)
        self.nc.scalar.activation(d, a, self.Act.Copy, bias=float(bias),
                                  scale=float(scale))
        return d

    def recip(self, a, dst=None, which=0):
        d = self._d(dst)
        mode = RECIP_MODE
        if mode == "mix":
            mode = "exact" if which == 2 else "acc"
        if mode == "exact":
            self.nc.vector.reciprocal(d, a)
        elif mode == "acc":
            scr = self.alloc()
            self.nc.vector.reciprocal_approx_accurate(d, a, scratch=scr)
            self.rel(scr)
        else:
            self.nc.vector.reciprocal_approx_fast(d, a)
        return d

    def recip_err(self, a, dst=None):
        # error-path 1/z: 51-ULP fast approx. Only perturbs the finite
        # reprojection distances at the fp32-noise level (~4e-3 px); the
        # error mean is +inf for the canonical inputs and dominated by
        # real misprojection for any realistic finite-error inputs.
        d = self._d(dst)
        self.nc.vector.reciprocal_approx_fast(d, a)
        return d

    def sqrt_accum(self, a, slot):
        # dedicated scratch: the sqrt output is a sink, and returning its
        # register to the free list would make the next DVE op inherit a
        # WAW dependency on this ACT op (a measured 2.2us stall)
        if not hasattr(self, "_sqrt_scr"):
            sqrtscr = self.pool.tile([P, F], self.f32, tag="sqrtscr")
            self._sqrt_scr = sqrtscr[:]
        self.nc.scalar.activation(self._sqrt_scr, a, self.Act.Sqrt,
                                  accum_out=self.ES[:, slot:slot + 1])

    def prefetch_prj(self):
        self.nc.sync.dma_start(
            self._prj_tile[:],
            self.prj_d.rearrange("(p f) c -> p (f c)", p=P))

    def pts_ready(self):
        self.nc.sync.dma_start(
            self.pts_d.rearrange("(p f) c -> p (f c)", p=P), self.OUT[:])

    def build(self, R, T, K1, d1, K2, d2):
        nc = self.nc
        with self._tile.TileContext(nc) as tc:
            with ExitStack() as ctx:
                self.pool = ctx.enter_context(
                    tc.tile_pool(name="pool", bufs=1))
                CAM = self.pool.tile([P, 2 * F], self.f32, tag="CAM")
                PRJ = self.pool.tile([P, 2 * F], self.f32, tag="PRJ")
                self.OUT = self.pool.tile([P, 3 * F], self.f32, tag="OUT")
                self.ES = self.pool.tile([P, 2], self.f32, tag="ES")
                nc.sync.dma_start(
                    CAM[:], self.cam_d.rearrange("(p f) c -> p (f c)", p=P))
                self._prj_tile = PRJ
                cam3 = CAM[:].rearrange("p (f c) -> p f c", c=2)
                prj3 = PRJ[:].rearrange("p (f c) -> p f c", c=2)
                out3 = self.OUT[:].rearrange("p (f c) -> p f c", c=3)
                self.u1, self.v1 = cam3[:, :, 0], cam3[:, :, 1]
                self.u2, self.v2 = prj3[:, :, 0], prj3[:, :, 1]
                self.outX = out3[:, :, 0]
                self.outY = out3[:, :, 1]
                self.outZ = out3[:, :, 2]
                for ap in (self.u1, self.v1, self.u2, self.v2,
                           self.outX, self.outY, self.outZ):
                    self._pinned.add(id(ap))
                _emit_pipeline(self, R, T, K1, d1, K2, d2)
                nc.sync.dma_start(self.esum_d, self.ES[:])
        nc.compile()
        return nc


def _get_program(R, T, K1, d1, K2, d2):
    key = _params_key(R, T, K1, d1, K2, d2)
    if key not in _CACHE:
        _CACHE[key] = _BassB().build(R, T, K1, d1, K2, d2)
    return _CACHE[key]


def kernel(camera_points, projector_points, R, T, camera_mtx, camera_dist,
           projector_mtx, projector_dist):
    camera_points = np.ascontiguousarray(camera_points, np.float32)
    projector_points = np.ascontiguousarray(projector_points, np.float32)
    R = np.asarray(R, np.float32)
    T = np.asarray(T, np.float32)
    K1 = np.asarray(camera_mtx, np.float32)
    d1 = np.asarray(camera_dist, np.float32)
    K2 = np.asarray(projector_mtx, np.float32)
    d2 = np.asarray(projector_dist, np.float32)

    B, N, _ = camera_points.shape
    total = B * N
    assert total == NCORES * PER_CORE, (B, N)

    nc = _get_program(R, T, K1, d1, K2, d2)

    cam_flat = camera_points.reshape(total, 2)
    prj_flat = projector_points.reshape(total, 2)
    in_maps = [
        {"cam": cam_flat[c * PER_CORE:(c + 1) * PER_CORE],
         "prj": prj_flat[c * PER_CORE:(c + 1) * PER_CORE]}
        for c in range(NCORES)
    ]

    from concourse.bass_utils import run_bass_kernel_spmd
    res = run_bass_kernel_spmd(nc, in_maps, list(range(NCORES))).results

    pts = np.concatenate([res[c]["pts"] for c in range(NCORES)], axis=0)
    pts = pts.reshape(B, N, 3)
    esum = np.stack([res[c]["esum"] for c in range(NCORES)])  # [8,128,2]
    es = esum.astype(np.float64).sum(axis=(0, 1))
    error = np.float32(0.5 * (es[0] / total + es[1] / total))
    return pts, error


# revision 69
# speedup vs baseline: 1.0144x; 1.0144x over previous
"""Trainium2 Bass kernel for nn_PlaneRefineBatch (stereo undistort + DLT
triangulation + reprojection error).

Contract: kernel(**inputs) takes the FULL inputs (camera_points [16,131072,2],
projector_points [16,131072,2], R [3,3], T [3], camera_mtx, camera_dist,
projector_mtx, projector_dist) and returns (pts [16,131072,3] f32, error f32
scalar) exactly like the reference.

Strategy: pure data-parallel over the 16-batch axis: 2 batches per core on 8
cores (SPMD, one Bass program). Per core 262144 points = [128 partitions x
2048 free] fp32 planes, whole-plane instructions. Calibration scalars are
baked into the program as immediates (program cached per calibration). Heavy
math on the Vector engine (fused scalar_tensor_tensor / ln_bwd_dx ops,
approx reciprocals); the error path's squares/sqrt+row-sum on the Scalar
engine.

The pipeline is emitted by `_emit_pipeline` against an abstract backend so
the exact same op sequence runs as Bass IR (class _BassB) or as a bit-
faithful numpy fp32 emulation (class _NpB in emulate.py) for fast numerical
iteration against the jax reference.

Math notes:
 - undistort: 2 fixed-point iterations match the reference's 5 within fp32
   noise (contraction ~0.03/iter); 1/radial = 1-g+g^2 exactly to O(g^3)~1e-11.
 - triangulation: T_MODE="exact" replicates the reference's einsum rounding
   (m3/m4/b rows, then ordered sums of products); "feat" uses cheaper linear
   combos of (s, x2, y2, x1, y1, sc).
 - solve: LDL (AtA+eps*I is SPD); reciprocal flavor per RECIP_MODE.
 - reprojection feeds only the scalar error (a mean, which overflows to +inf
   on degenerate points exactly like the reference does).
"""
import numpy as np
from contextlib import ExitStack

P = 128           # SBUF partitions
F = 2048          # free-dim elements per partition per plane
NCORES = 8
PER_CORE = P * F  # 262144 points per core
NREG = 17         # register planes of [P, F] fp32

UNDIST_ITERS = 2
T_MODE = "exact"       # "exact" (replicate einsum rounding) | "feat"
RECIP_MODE = "exact"   # LDL pivots: "exact" | "acc" (2 ULP) | "fast" (51 ULP)
# P_SLIM drops the reprojection's tangential terms + uses the fast 1/z.
# For the canonical setup_inputs() the error is +inf either way, but keep
# the faithful path so the error is also right for any finite-error inputs.
P_SLIM = False
# Reuse iteration-1's 1/radial in iteration 2 (it moves by ~1 ulp of x
# between iterations; verified against the reference in the emulator).
REUSE_G = False

_CACHE = {}
_CUSTOM_OPS = None


def _register_custom_ops():
    """Register fused DVE ops with the custom-DVE registry (HW-verified
    bitwise against the discrete chains they replace, incl. inf overflow):
      HORNER3: ((in0*s0 + s1)*in0 + imm2)*in0        (radial cubic)
      SQSQ:    (in0*in0 + in1*in1) + s0              (r2 / squared distance)
      UX2:     (in0 + in1*in1*s0)*s1                 ((r2 + 2x^2)*p)
    """
    global _CUSTOM_OPS
    if _CUSTOM_OPS is not None:
        return _CUSTOM_OPS
    from concourse import dve_ops
    from concourse.dve_spec import (Spec, Src0, Src1, C0, C1, C2, lower,
                                    _has_src1)
    from concourse.dve_uop import DveOpSpec
    f32 = np.float32

    def _ref_h3(in0, in1, s0, s1, imm2):
        t = (in0.astype(f32) * f32(s0)).astype(f32)
        t = (t + f32(s1)).astype(f32)
        t = (t * in0).astype(f32)
        t = (t + f32(imm2)).astype(f32)
        return (t * in0).astype(f32)

    def _ref_sqsq(in0, in1, s0, s1, imm2):
        t = (in0.astype(f32) * in0.astype(f32)).astype(f32)
        u = (in1.astype(f32) * in1.astype(f32)).astype(f32)
        return ((t + u).astype(f32) + f32(s0)).astype(f32)

    def _ref_ux2(in0, in1, s0, s1, imm2):
        t = (in1.astype(f32) * in1.astype(f32)).astype(f32)
        u = (t * f32(s0)).astype(f32)
        v = (in0.astype(f32) + u).astype(f32)
        return (v * f32(s1)).astype(f32)

    def _ref_tsts(in0, in1, s0, s1, imm2):
        t = (in0.astype(f32) * in1.astype(f32)).astype(f32)
        u = (t * f32(s0)).astype(f32)
        return (u + f32(s1)).astype(f32)

    defs = [
        ("HORNER3_ANT", Spec(body=((Src0 * C0 + C1) * Src0 + C2) * Src0,
                             reference=_ref_h3)),
        ("SQSQ_ANT", Spec(body=(Src0 * Src0 + Src1 * Src1) + C0,
                          reference=_ref_sqsq)),
        ("UX2_ANT", Spec(body=(Src0 + Src1 * Src1 * C0) * C1,
                         reference=_ref_ux2)),
        ("TSTS_ANT", Spec(body=Src0 * Src1 * C0 + C1,
                          reference=_ref_tsts)),
    ]
    ops = {}
    for name, spec in defs:
        if name in dve_ops._SUB_OPCODE_FOR_NAME:
            ops[name] = next(o for o in dve_ops.OPS if o.name == name)
            continue
        op = dve_ops.DveOp(name, spec, subdim=False, uops_sha={})
        row = max(dve_ops._SUB_OPCODE_FOR_NAME.values()) + 1
        assert row < 0x20
        dve_ops.OPS.append(op)
        dve_ops.CUSTOM_DVE_SPECS[name] = spec
        dve_ops._SUB_OPCODE_FOR_NAME[name] = row
        for ver in ("v3", "v4"):
            r = DveOpSpec(name=name, opcode=row, uops=lower(spec, ver=ver),
                          rd1_en=_has_src1(spec))
            op.uops_sha[ver] = r.sha(ver)
        ops[name] = op
    _CUSTOM_OPS = ops
    return ops


def _params_key(*arrs):
    return (UNDIST_ITERS, T_MODE, RECIP_MODE, P_SLIM) + tuple(a.tobytes() for a in arrs)


# --------------------------------------------------------------------------
# Shared pipeline emission (backend-agnostic)
# --------------------------------------------------------------------------

def _emit_undistort(B, u, v, K, dist, iters, split_head=False):
    fx, fy, cx, cy = (float(K[0, 0]), float(K[1, 1]),
                      float(K[0, 2]), float(K[1, 2]))
    k1, k2, p1, p2, k3 = [float(x) for x in dist]
    if split_head:
        # column-half normalize so the first half runs while the second
        # half of the input DMA is still streaming (value-identical)
        x0 = B.ts_halves(u, cx, 1.0 / fx, "subtract", "mult")
        y0 = B.ts_halves(v, cy, 1.0 / fy, "subtract", "mult")
    else:
        x0 = B.ts(u, cx, 1.0 / fx, "subtract", "mult")
        y0 = B.ts(v, cy, 1.0 / fy, "subtract", "mult")
    x, y = x0, y0
    ir_saved = None
    for it in range(iters):
        last = it == iters - 1
        r2 = B.sqsq(x, y, 0.0)              # x^2+y^2, fused
        xy = B.tt(x, y, "mult")
        ux = B.ux2(r2, x, 2.0, p2)          # p2*(r2+2x^2), fused
        uy = B.ux2(r2, y, 2.0, p1)          # p1*(r2+2y^2), fused
        if it > 0:
            B.rel(x, y)
        if REUSE_G and ir_saved is not None:
            ir = ir_saved
            B.rel(r2)
            wx = B.stt(xy, 2.0 * p1, ux, "mult", "add")
            wy = B.stt(xy, 2.0 * p2, uy, "mult", "add")
        else:
            g = B.horner3(r2, k3, k2, k1)  # ((k3*r2+k2)*r2+k1)*r2
            B.rel(r2)
            gsq = B.sq(g)   # ACT; covered by the DVE w-ops below
            wx = B.stt(xy, 2.0 * p1, ux, "mult", "add")
            wy = B.stt(xy, 2.0 * p2, uy, "mult", "add")
            ir = B.stt(gsq, 1.0, g, "add", "subtract")  # 1+g^2-g
            B.rel(g, gsq)
            if REUSE_G and not last:
                ir_saved = ir
        B.rel(ux, uy, xy)
        tx = B.tt(x0, wx, "subtract")
        ty = B.tt(y0, wy, "subtract")
        B.rel(wx, wy)
        x = B.tt(tx, ir, "mult")
        y = B.tt(ty, ir, "mult")
        B.rel(tx, ty)
        if not (REUSE_G and not last):
            B.rel(ir)
    B.rel(x0, y0)
    return x, y


def _emit_lin(B, terms, const, dst=None):
    """sum(c_i * t_i) + const via ln_bwd chains; drops zero coefficients."""
    terms = [(float(c), t) for c, t in terms if float(c) != 0.0]
    const = float(const)
    assert terms
    if len(terms) == 1:
        c0, t0 = terms[0]
        return B.ts(t0, c0, const, "mult", "add", dst=dst)
    terms.sort(key=lambda ct: -abs(ct[0]))
    (c0, t0), (c1, t1) = terms[0], terms[1]
    if len(terms) == 2:
        return B.lnb(t0, t1, -c1 / c0, -const / c0, c0, dst=dst)
    acc = B.lnb(t0, t1, -c1 / c0, 0.0, c0)
    for i, (ci, ti) in enumerate(terms[2:]):
        last = i == len(terms) - 3
        nxt = B.lnb(acc, ti, -ci, -const if last else 0.0, 1.0,
                    dst=(dst if last else None))
        B.rel(acc)
        acc = nxt
    return acc


def _emit_normal_eqs_feat(B, x1, y1, x2, y2, Rd, Td):
    """AtA/-Atb as linear combos of (s, x2, y2, x1, y1, sc)."""
    a_r, b_r, c_r = Rd[0], Rd[1], Rd[2]
    C2m = np.outer(c_r, c_r)
    CA = np.outer(a_r, c_r) + np.outer(c_r, a_r)
    CB = np.outer(b_r, c_r) + np.outer(c_r, b_r)
    Dm = np.outer(a_r, a_r) + np.outer(b_r, b_r)
    EPS = 1e-9
    sx2 = B.sq(x2); sy2 = B.sq(y2)
    s = B.tt(sx2, sy2, "add"); B.rel(sx2, sy2)
    sx1 = B.sq(x1); sy1 = B.sq(y1)
    sc = B.tt(sx1, sy1, "add"); B.rel(sx1, sy1)
    a00 = _emit_lin(B, [(C2m[0, 0], s), (-CA[0, 0], x2), (-CB[0, 0], y2)],
                    Dm[0, 0] + 1.0)
    a01 = _emit_lin(B, [(C2m[0, 1], s), (-CA[0, 1], x2), (-CB[0, 1], y2)],
                    Dm[0, 1])
    t11 = [(C2m[1, 1], s), (-CA[1, 1], x2), (-CB[1, 1], y2)]
    a11c = Dm[1, 1] + 1.0
    a11 = None if all(float(c) == 0.0 for c, _ in t11) else \
        _emit_lin(B, t11, a11c)
    a02 = _emit_lin(B, [(C2m[0, 2], s), (-CA[0, 2], x2), (-CB[0, 2], y2),
                        (-1.0, x1)], Dm[0, 2])
    a12 = _emit_lin(B, [(C2m[1, 2], s), (-CA[1, 2], x2), (-CB[1, 2], y2),
                        (-1.0, y1)], Dm[1, 2])
    a22 = _emit_lin(B, [(C2m[2, 2], s), (-CA[2, 2], x2), (-CB[2, 2], y2),
                        (1.0, sc)], Dm[2, 2] + EPS)
    B.rel(sc, x1, y1)
    q0 = _emit_lin(B, [(-Td[2] * c_r[0], s),
                       (c_r[0] * Td[0] + a_r[0] * Td[2], x2),
                       (c_r[0] * Td[1] + b_r[0] * Td[2], y2)],
                   -(a_r[0] * Td[0] + b_r[0] * Td[1]))
    q1 = _emit_lin(B, [(-Td[2] * c_r[1], s),
                       (c_r[1] * Td[0] + a_r[1] * Td[2], x2),
                       (c_r[1] * Td[1] + b_r[1] * Td[2], y2)],
                   -(a_r[1] * Td[0] + b_r[1] * Td[1]))
    q2 = _emit_lin(B, [(-Td[2] * c_r[2], s),
                       (c_r[2] * Td[0] + a_r[2] * Td[2], x2),
                       (c_r[2] * Td[1] + b_r[2] * Td[2], y2)],
                   -(a_r[2] * Td[0] + b_r[2] * Td[1]))
    B.rel(s, x2, y2)
    return a00, a01, a11, a11c, a02, a12, a22, q0, q1, q2


def _emit_normal_eqs_exact(B, x1, y1, x2, y2, Rd, Td):
    """Replicate the reference's einsum rounding: m3 = x2*R2 - R0,
    m4 = y2*R2 - R1, nb3 = -(x2*T2 - T0), nb4 = -(y2*T2 - T1), then AtA
    entries as ((m1.m1 + m2.m2) + m3.m3) + m4.m4 in that order, and
    q = -Atb (negation exact). Values are (tensor | ('c', float)) with
    constant folding, so zero rows of R/T cost nothing."""
    f32 = np.float32

    def is_c(v):
        return isinstance(v, tuple) and v[0] == 'c'

    def row(t, c, a):
        # t*c - a; c == 0 -> elementwise (+-0 - a) = -a exactly
        if float(c) == 0.0:
            return ('c', float(f32(-f32(a))))
        return B.ts(t, float(c), float(a), "mult", "subtract")

    def nrow(t, c, a):
        # -(t*c - a) = t*(-c) + a
        if float(c) == 0.0:
            return ('c', float(f32(a)))
        return B.ts(t, float(-c), float(a), "mult", "add")

    def mul(x, y):
        if is_c(x) and is_c(y):
            return ('c', float(f32(x[1]) * f32(y[1])))
        if is_c(y):
            x, y = y, x
        if is_c(x):
            if x[1] == 0.0:
                return ('c', 0.0)
            return B.ts(y, float(x[1]), 1.0, "mult", "mult")
        return B.tt(x, y, "mult")

    def sqv(x):
        if is_c(x):
            return ('c', float(f32(x[1]) * f32(x[1])))
        return B.sq(x)

    def add(x, y):
        if is_c(x) and is_c(y):
            return ('c', float(f32(x[1]) + f32(y[1])))
        if is_c(y):
            x, y = y, x
        if is_c(x):
            if x[1] == 0.0:
                return y
            return B.ts(y, float(x[1]), 1.0, "add", "mult")
        return B.tt(x, y, "add")

    def neg(x):
        if is_c(x):
            return ('c', -x[1])
        return B.ts(x, -1.0, 1.0, "mult", "mult")

    def relv(*vs):
        B.rel(*[v for v in vs if not is_c(v)])

    a_r, b_r, c_r = Rd[0], Rd[1], Rd[2]
    m3 = [row(x2, c_r[j], a_r[j]) for j in range(3)]
    m4 = [row(y2, c_r[j], b_r[j]) for j in range(3)]
    nb3 = nrow(x2, Td[2], Td[0])
    nb4 = nrow(y2, Td[2], Td[1])
    relv(x2, y2)  # fully consumed by the m/nb rows

    def fin(res, temps):
        seen = {id(res)}
        for v in temps:
            if v is None or is_c(v) or id(v) in seen:
                continue
            seen.add(id(v))
            B.rel(v)
        return res

    def entry(base_c, p, q):
        # ((base_c + p) + q) with reference rounding order
        t = add(('c', base_c), p)
        return fin(add(t, q), [p, q, t])

    def entry_neg(xb, p, q):
        # ((-xb + p) + q); xb is a live tensor (not freed here)
        if is_c(p):
            assert not is_c(q)
            if p[1] == 0.0:
                return B.tt(q, xb, "subtract")  # (-xb + 0) + q == q - xb
            t = B.ts(xb, -1.0, float(p[1]), "mult", "add")  # -xb + p
            return fin(add(t, q), [q, t])
        t = B.tt(p, xb, "subtract")         # == -xb + p
        return fin(add(t, q), [p, q, t])

    # hoist ACT squares just enough to overlap the DVE products below
    # without blowing the register budget
    s3x, s4x = sqv(m3[0]), sqv(m4[0])
    s3y, s4y = sqv(m3[1]), sqv(m4[1])
    a01 = entry(0.0, mul(m3[0], m3[1]), mul(m4[0], m4[1]))
    a02 = entry_neg(x1, mul(m3[0], m3[2]), mul(m4[0], m4[2]))
    a12 = entry_neg(y1, mul(m3[1], m3[2]), mul(m4[1], m4[2]))
    sx1 = sqv(x1); sy1 = sqv(y1)
    relv(x1, y1)  # dead once their squares are issued
    a00 = entry(1.0, s3x, s4x)
    a11 = entry(1.0, s3y, s4y)
    s3z, s4z = sqv(m3[2]), sqv(m4[2])
    sc = add(sx1, sy1); relv(sx1, sy1)
    t = add(sc, s3z); relv(sc, s3z)
    u = add(t, s4z); relv(s4z, t)
    a22 = B.ts(u, 1e-9, 1.0, "add", "mult")
    relv(u)
    qs = []
    for j in range(3):
        pj = mul(m3[j], nb3)
        qj = mul(m4[j], nb4)
        qs.append(fin(add(pj, qj), [pj, qj]))
    relv(*m3, *m4, nb3, nb4)
    a11c = None
    if is_c(a11):
        a11c = a11[1]
        a11 = None
    for v in (a00, a01, a02, a12, a22, qs[0], qs[1], qs[2]):
        assert not is_c(v), "unexpected constant normal-equation entry"
    return a00, a01, a11, a11c, a02, a12, a22, qs[0], qs[1], qs[2]


def _emit_solve_ldl(B, a00, a01, a11, a11c, a02, a12, a22, q0, q1, q2):
    """LDL solve of (AtA)x = q, writing x into B.outX/outY/outZ."""
    i0 = B.recip(a00, which=0)
    B.rel(a00)
    l10 = B.tt(a01, i0, "mult")
    l20 = B.tt(a02, i0, "mult")
    t3 = B.tt(l20, a02, "mult")
    B.rel(a02)
    # NOTE: fusing this into a custom TSTS op (a*b*c0+c1) coincided with an
    # NRT_EXEC_UNIT_UNRECOVERABLE device crash -- left unfused.
    t = B.tt(l10, a01, "mult")
    if a11 is None:
        d1v = B.ts(t, -1.0, float(a11c), "mult", "add")
    else:
        d1v = B.tt(a11, t, "subtract")
        B.rel(a11)
    B.rel(t)
    t2 = B.tt(l20, a01, "mult")
    B.rel(a01)
    a12p = B.tt(a12, t2, "subtract")
    B.rel(a12, t2)
    i1 = B.recip(d1v, which=1)
    B.rel(d1v)
    l21 = B.tt(a12p, i1, "mult")
    t4 = B.tt(l21, a12p, "mult")
    B.rel(a12p)
    t5 = B.tt(t3, t4, "add")
    B.rel(t3, t4)
    d2v = B.tt(a22, t5, "subtract")
    B.rel(a22, t5)
    i2 = B.recip(d2v, which=2)
    B.rel(d2v)
    t6 = B.tt(l10, q0, "mult")
    c1 = B.tt(q1, t6, "subtract")
    B.rel(q1, t6)
    t7 = B.tt(l20, q0, "mult")
    t8 = B.tt(q2, t7, "subtract")
    B.rel(q2, t7)
    t9 = B.tt(l21, c1, "mult")
    c2 = B.tt(t8, t9, "subtract")
    B.rel(t8, t9)
    e0 = B.tt(q0, i0, "mult")
    B.rel(q0, i0)
    e1 = B.tt(c1, i1, "mult")
    B.rel(c1, i1)
    B.tt(c2, i2, "mult", dst=B.outZ)
    B.rel(c2, i2)
    t10 = B.tt(l21, B.outZ, "mult")
    B.rel(l21)
    B.tt(e1, t10, "subtract", dst=B.outY)
    B.rel(e1, t10)
    t11 = B.tt(l10, B.outY, "mult")
    B.rel(l10)
    t12 = B.tt(e0, t11, "subtract")
    B.rel(e0, t11)
    t13 = B.tt(l20, B.outZ, "mult")
    B.rel(l20)
    B.tt(t12, t13, "subtract", dst=B.outX)
    B.rel(t12, t13)


def _emit_reproject_view(B, X, Y, Z, uo, vo, K, dist, slot, filler=None):
    fx, fy, cx, cy = (float(K[0, 0]), float(K[1, 1]),
                      float(K[0, 2]), float(K[1, 2]))
    k1, k2, p1, p2, k3 = [float(x) for x in dist]
    iz = B.recip_err(Z)
    x = B.tt(X, iz, "mult")
    y = B.tt(Y, iz, "mult")
    B.rel(iz)
    r2 = B.sqsq(x, y, 0.0)
    if not P_SLIM:
        xy = B.tt(x, y, "mult")
        ux = B.ux2(r2, x, 2.0, p2)
        uy = B.ux2(r2, y, 2.0, p1)
    g = B.horner3(r2, k3, k2, k1)
    B.rel(r2)
    sx = B.stt(g, 1.0, x, "add", "mult")   # radial*x
    sy = B.stt(g, 1.0, y, "add", "mult")
    B.rel(g, x, y)
    if P_SLIM:
        xd, yd = sx, sy
    else:
        t1 = B.stt(xy, 2.0 * p1, ux, "mult", "add")
        t2 = B.stt(xy, 2.0 * p2, uy, "mult", "add")
        B.rel(ux, uy, xy)
        xd = B.tt(sx, t1, "add")
        yd = B.tt(sy, t2, "add")
        B.rel(sx, sy, t1, t2)
    du = B.lnb(xd, uo, 1.0 / fx, -cx / fx, fx)  # fx*xd+cx-uo
    dv = B.lnb(yd, vo, 1.0 / fy, -cy / fy, fy)
    B.rel(xd, yd)
    out = filler() if filler is not None else None
    # (du^2 + dv^2) + eps fused -- matches the reference's sum-then-eps order
    d2s = B.sqsq(du, dv, 1e-9)
    B.rel(du, dv)
    B.sqrt_accum(d2s, slot)
    B.rel(d2s)
    return out


def _emit_pipeline(B, R, T, K1, d1, K2, d2):
    Rd = np.asarray(R, np.float64)
    Td = np.asarray(T, np.float64)
    # NOTE: column-splitting the input DMA + first normalize (to shave the
    # ~3 us DMA head) modeled fine but crashed real HW with
    # NRT_EXEC_UNIT_UNRECOVERABLE -- do not re-enable split_head without
    # hardware validation.
    x1, y1 = _emit_undistort(B, B.u1, B.v1, K1, d1, UNDIST_ITERS)
    B.prefetch_prj()   # projector input DMA off the camera DMA's head
    x2, y2 = _emit_undistort(B, B.u2, B.v2, K2, d2, UNDIST_ITERS)
    if T_MODE == "exact":
        eqs = _emit_normal_eqs_exact(B, x1, y1, x2, y2, Rd, Td)
    else:
        eqs = _emit_normal_eqs_feat(B, x1, y1, x2, y2, Rd, Td)
    _emit_solve_ldl(B, *eqs)
    B.pts_ready()

    trivial_py = (Rd[1, 0] == 0.0 and Rd[1, 2] == 0.0
                  and Rd[1, 1] == 1.0 and Td[1] == 0.0)

    def transform():
        # projector-frame transform; emitted as DVE filler inside the
        # camera view's ACT-square window
        px = _emit_lin(B, [(Rd[0, 0], B.outX), (Rd[0, 1], B.outY),
                           (Rd[0, 2], B.outZ)], Td[0])
        py = B.outY if trivial_py else _emit_lin(
            B, [(Rd[1, 0], B.outX), (Rd[1, 1], B.outY), (Rd[1, 2], B.outZ)],
            Td[1])
        pz = _emit_lin(B, [(Rd[2, 0], B.outX), (Rd[2, 1], B.outY),
                           (Rd[2, 2], B.outZ)], Td[2])
        return px, py, pz

    px, py, pz = _emit_reproject_view(B, B.outX, B.outY, B.outZ,
                                      B.u1, B.v1, K1, d1, 0,
                                      filler=transform)
    _emit_reproject_view(B, px, py, pz, B.u2, B.v2, K2, d2, 1)
    B.rel(px, pz)
    if not trivial_py:
        B.rel(py)


# --------------------------------------------------------------------------
# Bass backend
# --------------------------------------------------------------------------

class _BassB:
    def __init__(self):
        from concourse import mybir, tile
        from concourse.bacc import Bacc
        self.mybir = mybir
        self.f32 = mybir.dt.float32
        self.Alu = mybir.AluOpType
        self.Act = mybir.ActivationFunctionType
        nc = Bacc("TRN2", target_bir_lowering=False, debug=False,
                  num_devices=NCORES)
        self.nc = nc
        self.cam_d = nc.dram_tensor("cam", [PER_CORE, 2], self.f32,
                                    kind="ExternalInput").ap()
        self.prj_d = nc.dram_tensor("prj", [PER_CORE, 2], self.f32,
                                    kind="ExternalInput").ap()
        self.pts_d = nc.dram_tensor("pts", [PER_CORE, 3], self.f32,
                                    kind="ExternalOutput").ap()
        self.esum_d = nc.dram_tensor("esum", [P, 2], self.f32,
                                     kind="ExternalOutput").ap()
        self._tile = tile
        self._free = []
        self._nreg = 0
        self._pinned = set()

    def alloc(self):
        if self._free:
            # FIFO: reuse the OLDEST-released plane. LIFO handed the most
            # recently released register (often still being read by an
            # in-flight ACT op, e.g. d2s under the sqrt) to the next DVE
            # op, creating a 2.2us WAR stall.
            return self._free.pop(0)
        t = self.pool.tile([P, F], self.f32, tag=f"r{self._nreg}")
        self._nreg += 1
        assert self._nreg <= NREG, "register budget exceeded"
        return t[:]

    def rel(self, *aps):
        for ap in aps:
            if ap is not None and id(ap) not in self._pinned:
                self._free.append(ap)

    def _d(self, dst):
        return dst if dst is not None else self.alloc()

    def tt(self, a, b, op, dst=None):
        d = self._d(dst)
        self.nc.vector.tensor_tensor(d, a, b, getattr(self.Alu, op))
        return d

    def ts(self, a, s1, s2, op0, op1, dst=None):
        d = self._d(dst)
        self.nc.vector.tensor_scalar(d, a, float(s1), float(s2),
                                     getattr(self.Alu, op0),
                                     getattr(self.Alu, op1))
        return d

    def ts_halves(self, a, s1, s2, op0, op1, dst=None):
        # same values as ts(), emitted as two column-half ops so the first
        # half can start as soon as the first input-DMA chunk lands
        d = self._d(dst)
        h = F // 2
        for lo, hi in ((0, h), (h, F)):
            self.nc.vector.tensor_scalar(d[:, lo:hi], a[:, lo:hi],
                                         float(s1), float(s2),
                                         getattr(self.Alu, op0),
                                         getattr(self.Alu, op1))
        return d

    def stt(self, a, s, b, op0, op1, dst=None):
        d = self._d(dst)
        self.nc.vector.scalar_tensor_tensor(d, a, float(s), b,
                                            getattr(self.Alu, op0),
                                            getattr(self.Alu, op1))
        return d

    def lnb(self, a, b, s0, s1, imm2, dst=None):
        d = self._d(dst)
        self.nc.vector.ln_bwd_dx(d, a, b, float(s0), float(s1), float(imm2))
        return d

    def horner3(self, a, c0, c1, c2, dst=None):
        # ((a*c0 + c1)*a + c2)*a in one fused DVE op
        d = self._d(dst)
        self.nc.vector._custom_dve(_register_custom_ops()["HORNER3_ANT"],
                                   out=d, in0=a,
                                   s0=float(c0), s1=float(c1), imm2=float(c2))
        return d

    def tsts(self, a, b, c0, c1, dst=None):
        # a*b*c0 + c1 in one fused DVE op
        d = self._d(dst)
        self.nc.vector._custom_dve(_register_custom_ops()["TSTS_ANT"],
                                   out=d, in0=a, in1=b,
                                   s0=float(c0), s1=float(c1))
        return d

    def sqsq(self, a, b, c0, dst=None):
        # (a*a + b*b) + c0 in one fused DVE op
        d = self._d(dst)
        self.nc.vector._custom_dve(_register_custom_ops()["SQSQ_ANT"],
                                   out=d, in0=a, in1=b, s0=float(c0))
        return d

    def ux2(self, r2, x, c0, c1, dst=None):
        # (r2 + x*x*c0)*c1 in one fused DVE op
        d = self._d(dst)
        self.nc.vector._custom_dve(_register_custom_ops()["UX2_ANT"],
                                   out=d, in0=r2, in1=x,
                                   s0=float(c0), s1=float(c1))
        return d

    def sq(self, a, dst=None):
        # ACT Square is HW-verified bit-exact vs IEEE x*x -> offload the
        # (otherwise DVE-bound) squares to the idle Scalar engine
        return self.sq_act(a, dst=dst)

    def sq_act(self, a, dst=None):
        d = self._d(dst)
        self.nc.scalar.activation(d, a, self.Act.Square)
        return d

    def affine_act(self, a, scale, bias, dst=None):
        d = self._d(dst)
        self.nc.scalar.activation(d, a, self.Act.Copy, bias=float(bias),
                                  scale=float(scale))
        return d

    def recip(self, a, dst=None, which=0):
        d = self._d(dst)
        mode = RECIP_MODE
        if mode == "mix":
            mode = "exact" if which == 2 else "acc"
        if mode == "exact":
            self.nc.vector.reciprocal(d, a)
        elif mode == "acc":
            scr = self.alloc()
            self.nc.vector.reciprocal_approx_accurate(d, a, scratch=scr)
            self.rel(scr)
        else:
            self.nc.vector.reciprocal_approx_fast(d, a)
        return d

    def recip_err(self, a, dst=None):
        # error-path 1/z: 51-ULP fast approx. Only perturbs the finite
        # reprojection distances at the fp32-noise level (~4e-3 px); the
        # error mean is +inf for the canonical inputs and dominated by
        # real misprojection for any realistic finite-error inputs.
        d = self._d(dst)
        self.nc.vector.reciprocal_approx_fast(d, a)
        return d

    def sqrt_accum(self, a, slot):
        # dedicated scratch: the sqrt output is a sink, and returning its
        # register to the free list would make the next DVE op inherit a
        # WAW dependency on this ACT op (a measured 2.2us stall)
        if not hasattr(self, "_sqrt_scr"):
            sqrtscr = self.pool.tile([P, F], self.f32, tag="sqrtscr")
            self._sqrt_scr = sqrtscr[:]
        self.nc.scalar.activation(self._sqrt_scr, a, self.Act.Sqrt,
                                  accum_out=self.ES[:, slot:slot + 1])

    def prefetch_prj(self):
        self.nc.sync.dma_start(
            self._prj_tile[:],
            self.prj_d.rearrange("(p f) c -> p (f c)", p=P))

    def pts_ready(self):
        self.nc.sync.dma_start(
            self.pts_d.rearrange("(p f) c -> p (f c)", p=P), self.OUT[:])

    def build(self, R, T, K1, d1, K2, d2):
        nc = self.nc
        with self._tile.TileContext(nc) as tc:
            with ExitStack() as ctx:
                self.pool = ctx.enter_context(
                    tc.tile_pool(name="pool", bufs=1))
                CAM = self.pool.tile([P, 2 * F], self.f32, tag="CAM")
                PRJ = self.pool.tile([P, 2 * F], self.f32, tag="PRJ")
                self.OUT = self.pool.tile([P, 3 * F], self.f32, tag="OUT")
                self.ES = self.pool.tile([P, 2], self.f32, tag="ES")
                nc.sync.dma_start(
                    CAM[:], self.cam_d.rearrange("(p f) c -> p (f c)", p=P))
                self._prj_tile = PRJ
                cam3 = CAM[:].rearrange("p (f c) -> p f c", c=2)
                prj3 = PRJ[:].rearrange("p (f c) -> p f c", c=2)
                out3 = self.OUT[:].rearrange("p (f c) -> p f c", c=3)
                self.u1, self.v1 = cam3[:, :, 0], cam3[:, :, 1]
                self.u2, self.v2 = prj3[:, :, 0], prj3[:, :, 1]
                self.outX = out3[:, :, 0]
                self.outY = out3[:, :, 1]
                self.outZ = out3[:, :, 2]
                for ap in (self.u1, self.v1, self.u2, self.v2,
                           self.outX, self.outY, self.outZ):
                    self._pinned.add(id(ap))
                _emit_pipeline(self, R, T, K1, d1, K2, d2)
                nc.sync.dma_start(self.esum_d, self.ES[:])
        nc.compile()
        return nc


def _get_program(R, T, K1, d1, K2, d2):
    key = _params_key(R, T, K1, d1, K2, d2)
    if key not in _CACHE:
        _CACHE[key] = _BassB().build(R, T, K1, d1, K2, d2)
    return _CACHE[key]


def kernel(camera_points, projector_points, R, T, camera_mtx, camera_dist,
           projector_mtx, projector_dist):
    camera_points = np.ascontiguousarray(camera_points, np.float32)
    projector_points = np.ascontiguousarray(projector_points, np.float32)
    R = np.asarray(R, np.float32)
    T = np.asarray(T, np.float32)
    K1 = np.asarray(camera_mtx, np.float32)
    d1 = np.asarray(camera_dist, np.float32)
    K2 = np.asarray(projector_mtx, np.float32)
    d2 = np.asarray(projector_dist, np.float32)

    B, N, _ = camera_points.shape
    total = B * N
    assert total == NCORES * PER_CORE, (B, N)

    nc = _get_program(R, T, K1, d1, K2, d2)

    cam_flat = camera_points.reshape(total, 2)
    prj_flat = projector_points.reshape(total, 2)
    in_maps = [
        {"cam": cam_flat[c * PER_CORE:(c + 1) * PER_CORE],
         "prj": prj_flat[c * PER_CORE:(c + 1) * PER_CORE]}
        for c in range(NCORES)
    ]

    from concourse.bass_utils import run_bass_kernel_spmd
    res = run_bass_kernel_spmd(nc, in_maps, list(range(NCORES))).results

    pts = np.concatenate([res[c]["pts"] for c in range(NCORES)], axis=0)
    pts = pts.reshape(B, N, 3)
    esum = np.stack([res[c]["esum"] for c in range(NCORES)])  # [8,128,2]
    es = esum.astype(np.float64).sum(axis=(0, 1))
    error = np.float32(0.5 * (es[0] / total + es[1] / total))
    return pts, error
